# revision 13
# baseline (speedup 1.0000x reference)
"""LongcatMoe Trainium2 kernel — expert-parallel sparse MoE across 8 NeuronCores.

Strategy (expert-parallel, per the sharding hint):
  - Host computes the tiny router (fp64 softmax/top-k) and dispatches tokens
    by top-k expert id: core e receives the tokens routed to expert e (padded
    to capacity C=256, capacity factor 1.0; overflow falls back to an exact
    host computation), plus expert e's weights.
  - Each core runs the silu-gated MLP for its expert on its token block.
  - Host combines: out[tok] += gate_weight * y, plus the zero-expert
    (identity) term zero_w[t] * x[t].

v5 kernel: fp8 DoubleRow phase 1 + bf16 phase 2.
  Phase 1 (gate/up) runs "flipped": the stationary PE operand is an fp8
  x-block [128, 2, 128] (DoubleRow packs K=256 contraction rows), the moving
  operand streams fp8 gate/up weights [128, 2, 512] (FD=512) at 2 rows/cycle
  (measured 216 ns/MM warm = 2x the bf16 rate), producing mid in [C, I]
  layout in PSUM. Scales keep values in e4m3's normal range, all exact
  powers of two:
      x *= 2^4,  w_gate/w_up *= 2^7   (fp8)
      silu via activation(scale=2^-11) recovers the true gate
      the up-path 2^11 factor is folded into w_down (*= 2^-11, bf16, exact)
  mid [C, I] is flipped to [I, C]: units 1-3 by DMA XBAR transposes on the
  (slow, ~50 GB/s but idle) ACT ring, hidden under phase 1; the last unit
  by PE transposes (identity matmul) so the ACT latency never sits between
  the phases. Phase 2 (down) is bf16: stationary w_down blocks, stream mid
  [128, 256], fp32 PSUM, y written in 4-tile groups.

DMA design (v2-v4 trace analysis): the ACT HWDGE ring moves only ~50 GB/s
vs SP's ~230-430 GB/s, and the global DMA-semaphore pool is ~11; when the
transfer count exceeds it, await-space waits chain new transfers behind
whichever transfer recycled the semaphore — in v2-v4 that serialized the
w_down stream behind mid transposes that only fire late in phase 1,
starving phase 2 and oscillating the HAM clock gate. So: few, large
transfers (12 total), all bulk on SP in exact consumption order, created
first so semaphores are assigned in stream order.

Host-side layouts (per-partition contiguous for every device DMA):
  x8   [128, KH, 2, C]  fp8     x8[p,kh,o,c]   = x[idx[c], kh*256+o*128+p]*2^4
  wgu8 [2, 128, KH, 2, 2, 512] fp8
                        wgu8[ih,p,kh,g,o,n] = w{g}[kh*256+o*128+p, ih*512+n]*2^7
  wd   [2, 128, 8, IO, 128] bf16  wd[s,p,k,j,c] = w_down[j*128+p,
                                                   (8s+k)*128+c]*2^-11
  y    [4, 128, 4, C] bf16 output; host reassembles [H, C]
"""

import os

import numpy as np
import ml_dtypes

T, H, I, E, Z, TOPK = 1024, 2048, 1024, 8, 8, 4
ROUTED_SCALING = 1.0
N_CORES = 8
P = 128
HO = H // P   # 16
IO = I // P   # 8
KH = H // 256  # 8 DoubleRow k-groups
C = 256       # per-expert token capacity (capacity factor 1.0; overflow → host)
NWARM = 28    # PE warmup matmuls (bridge HAM until first real matmul ~11.3us)

XS = 2.0 ** 4   # x fp8 pre-scale
WS = 2.0 ** 7   # w_gate/w_up fp8 pre-scale
DS = 2.0 ** -11  # inverse of XS*WS; silu scale and w_down fold

_PROGRAM = None
LAST_RESULTS = None  # BassKernelResults of the most recent run (for test harness)


def _build_program():
    import concourse.mybir as mybir
    import concourse.tile as tile
    from concourse import bacc
    from concourse.masks import make_identity

    f32 = mybir.dt.float32
    bf16 = mybir.dt.bfloat16
    fp8 = mybir.dt.float8e4
    SILU = mybir.ActivationFunctionType.Silu
    DR = mybir.MatmulPerfMode.DoubleRow

    nc = bacc.Bacc(
        "TRN2",
        target_bir_lowering=False,
        debug=False,
        enable_asserts=False,
        num_devices=N_CORES,
    )
    x8 = nc.dram_tensor("x8", [P, KH, 2, C], fp8, kind="ExternalInput").ap()
    wgu8 = nc.dram_tensor("wgu8", [2, P, KH, 2, 2, 512], fp8,
                          kind="ExternalInput").ap()
    wd = nc.dram_tensor("wd", [2, P, HO // 2, IO, P], bf16,
                        kind="ExternalInput").ap()
    y = nc.dram_tensor("y", [4, P, 4, C], bf16, kind="ExternalOutput").ap()

    with tile.TileContext(nc) as tc:
        with (
            tc.tile_pool(name="px", bufs=1) as px,
            tc.tile_pool(name="pwgu", bufs=4) as pwgu,
            tc.tile_pool(name="pwd", bufs=2) as pwd,
            tc.tile_pool(name="pmidc", bufs=4) as pmidc,
            tc.tile_pool(name="pmidi", bufs=1) as pmidi,
            tc.tile_pool(name="psg", bufs=2) as psg,
            tc.tile_pool(name="pid", bufs=1) as pid,
            tc.tile_pool(name="py", bufs=4) as py,
            tc.tile_pool(name="pwrm", bufs=1) as pwrm,
            tc.tile_pool(name="ppg", bufs=2, space="PSUM") as ppg,
            tc.tile_pool(name="ppu", bufs=2, space="PSUM") as ppu,
            tc.tile_pool(name="ppd", bufs=2, space="PSUM") as ppd,
            tc.tile_pool(name="ppt", bufs=1, space="PSUM") as ppt,
            tc.tile_pool(name="ppw", bufs=1, space="PSUM") as ppw,
        ):
            # PE warmup: keep the tensor engine busy while the head DMAs land
            # so the HAM clock-gate reaches 2.4 GHz by the first real matmul
            # (and stays there: no PE gap may exceed ~3.4us).
            wtile = pwrm.tile([P, C], bf16)
            nc.vector.memset(wtile[:], 0.0)
            pwm = ppw.tile([P, C], f32)
            for w in range(NWARM):
                nc.tensor.matmul(pwm[:], wtile[:, :P], wtile[:],
                                 start=(w == 0), stop=(w == NWARM - 1))

            ident = pid.tile([P, P], bf16)
            make_identity(nc, ident[:])

            xt = px.tile([P, KH, 2, C], fp8)
            wgu_t = [[pwgu.tile([P, KH // 2, 2, 2, 512], fp8,
                                name=f"wgu{ih}_{q}", tag="wgu")
                      for q in range(2)] for ih in range(2)]
            wd_t = [pwd.tile([P, HO // 2, IO, P], bf16, name=f"wd{s}",
                             tag="wd") for s in range(2)]
            mid_i = pmidi.tile([P, IO, C], bf16)

            # Bulk input DMAs: ALL on the SP ring, in exact consumption
            # order, sem-assigned first (high_priority): 7 transfers.
            with tc.high_priority():
                nc.sync.dma_start(xt[:], x8[:])
                for ih in range(2):
                    for q in range(2):
                        kl = q * (KH // 2)
                        nc.sync.dma_start(wgu_t[ih][q][:],
                                          wgu8[ih][:, kl:kl + KH // 2])
                for s in range(2):
                    nc.sync.dma_start(wd_t[s][:], wd[s])

            # Phase 1 (flipped, fp8 DoubleRow): for unit (cb, ih), psum
            # [c=128, i=512] accumulates over KH k-groups of 256 h-rows.
            # c-inner unit order so each wgu half is consumed by two units
            # while DMA fetches the next half.
            units = [(cb, ih) for ih in range(2) for cb in range(2)]
            for u, (cb, ih) in enumerate(units):
                pg = ppg.tile([P, 512], f32)
                pu = ppu.tile([P, 512], f32)
                for kh in range(KH):
                    wt = wgu_t[ih][kh // (KH // 2)]
                    ko = kh % (KH // 2)
                    xst = xt[:, kh, :, cb * P:(cb + 1) * P]
                    nc.tensor.matmul(
                        pg[:], xst, wt[:, ko, 0, :, :],
                        start=(kh == 0), stop=(kh == KH - 1),
                        perf_mode=DR,
                    )
                    nc.tensor.matmul(
                        pu[:], xst, wt[:, ko, 1, :, :],
                        start=(kh == 0), stop=(kh == KH - 1),
                        perf_mode=DR,
                    )
                sg = psg.tile([P, 512], f32)
                nc.scalar.activation(sg[:], pg[:], SILU, scale=DS)
                midc = pmidc.tile([P, 512], bf16)
                nc.vector.tensor_mul(out=midc[:], in0=sg[:], in1=pu[:])
                if u < len(units) - 1:
                    # Flip [c,i]->[i,c] via DMA XBAR transpose on the ACT
                    # ring (slow but hidden under phase 1).
                    nc.scalar.dma_start_transpose(
                        mid_i[:, ih * 4:(ih + 1) * 4, cb * P:(cb + 1) * P],
                        midc[:])
                else:
                    # Last unit: the ACT ring's latency would delay phase 2;
                    # transpose on the (idle, warm) PE instead.
                    for m in range(4):
                        pt = ppt.tile([P, P], bf16)
                        nc.tensor.transpose(
                            pt[:], midc[:, m * P:(m + 1) * P], ident[:])
                        nc.vector.tensor_copy(
                            out=mid_i[:, ih * 4 + m, cb * P:(cb + 1) * P],
                            in_=pt[:])

            # Phase 2 (bf16): y[k] = sum_j Wd[j, k].T @ mid[j] in [H, C];
            # y tiles written out in groups of 4 on the SP ring.
            for kq in range(4):
                ty = py.tile([P, 4, C], bf16)
                for sub in range(4):
                    k = kq * 4 + sub
                    pd = ppd.tile([P, C], f32)
                    for j in range(IO):
                        nc.tensor.matmul(
                            pd[:], wd_t[k // (HO // 2)][:, k % (HO // 2), j, :],
                            mid_i[:, j, :],
                            start=(j == 0), stop=(j == IO - 1),
                        )
                    nc.vector.tensor_copy(out=ty[:, sub, :], in_=pd[:])
                nc.sync.dma_start(y[kq], ty[:])

    nc.compile()
    return nc


def _route(x, router_w, corr_bias):
    """fp64 router: returns (topk_idx [T,K], topk_w [T,K])."""
    xl = x.astype(np.float64)
    logits = xl @ router_w.astype(np.float64).T
    logits -= logits.max(axis=1, keepdims=True)
    p = np.exp(logits)
    p /= p.sum(axis=1, keepdims=True)
    sel = p + corr_bias.astype(np.float64)
    topk_idx = np.argsort(-sel, axis=1, kind="stable")[:, :TOPK]
    topk_w = np.take_along_axis(p, topk_idx, axis=1) * ROUTED_SCALING
    return topk_idx, topk_w


def kernel(hidden_states, router_w, corr_bias, w_gate, w_up, w_down):
    global _PROGRAM, LAST_RESULTS
    x = np.asarray(hidden_states, dtype=np.float32)
    router_w = np.asarray(router_w, dtype=np.float32)
    corr_bias = np.asarray(corr_bias, dtype=np.float32)
    w_gate = np.asarray(w_gate, dtype=np.float32)
    w_up = np.asarray(w_up, dtype=np.float32)
    w_down = np.asarray(w_down, dtype=np.float32)

    topk_idx, topk_w = _route(x, router_w, corr_bias)
    routed = topk_idx < E
    zero_w = (topk_w * (~routed)).sum(axis=1)  # [T] fp64

    bf = ml_dtypes.bfloat16
    e4 = ml_dtypes.float8_e4m3

    # Dispatch: token list + gate weight per expert; overflow beyond C
    # falls back to an exact host computation.
    idx_list, w_list, overflow = [], [], []
    for e in range(E):
        toks, kpos = np.nonzero(topk_idx == e)
        we = topk_w[toks, kpos]
        if len(toks) > C:
            overflow.append((e, toks[C:], we[C:]))
            toks, we = toks[:C], we[:C]
        idx_list.append(toks)
        w_list.append(we)

    in_maps = []
    for e in range(E):
        toks = idx_list[e]
        n = len(toks)
        xg = np.zeros((C, H), dtype=np.float32)
        xg[:n] = x[toks]
        # x8[p, kh, o, c] = x[c, kh*256+o*128+p] * XS
        x8d = np.ascontiguousarray(
            (xg * XS).astype(e4).reshape(C, KH, 2, P).transpose(3, 1, 2, 0))
        # wgu8[ih, p, kh, g, o, n] = w{g}[kh*256+o*128+p, ih*512+n] * WS
        wg8 = (w_gate[e] * WS).astype(e4)
        wu8 = (w_up[e] * WS).astype(e4)
        # [g, H, I] -> [g, kh, o, p, ih, n] -> [ih, p, kh, g, o, n]
        wgu_s = np.stack([wg8, wu8], axis=0).reshape(2, KH, 2, P, 2, 512)
        wgud = np.ascontiguousarray(wgu_s.transpose(4, 3, 1, 0, 2, 5))
        # wd[s, p, k, j, c] = w_down[j*128+p, (8s+k)*128+c] * DS
        wdd = np.ascontiguousarray(
            (w_down[e] * DS).astype(bf).reshape(IO, P, 2, HO // 2, P)
            .transpose(2, 1, 3, 0, 4))
        in_maps.append({"x8": x8d, "wgu8": wgud, "wd": wdd})

    if _PROGRAM is None:
        _PROGRAM = _build_program()

    from concourse.bass_utils import run_bass_kernel_spmd

    kw = {}
    if os.environ.get("MOE_KERNEL_TRACE", "") == "1":
        kw = dict(trace=True, trace_cores=list(range(N_CORES)))
    res = run_bass_kernel_spmd(
        _PROGRAM, in_maps, core_ids=list(range(N_CORES)), **kw)
    LAST_RESULTS = res

    out = np.zeros((T, H), dtype=np.float64)
    for e in range(E):
        n = len(idx_list[e])
        if n:
            yr = res.results[e]["y"]  # [4, P, 4, C] bf16
            ye = yr.transpose(0, 2, 1, 3).reshape(H, C)
            out[idx_list[e]] += (w_list[e][:, None]
                                 * ye[:, :n].T.astype(np.float64))
    for e, toks, ws in overflow:
        xt = x[toks].astype(np.float64)
        g = xt @ w_gate[e].astype(np.float64)
        u = xt @ w_up[e].astype(np.float64)
        mid = (g / (1.0 + np.exp(-g))) * u
        out[toks] += ws[:, None] * (mid @ w_down[e].astype(np.float64))
    out += zero_w[:, None] * x.astype(np.float64)
    return out.astype(np.float32)


# revision 14
# speedup vs baseline: 1.3516x; 1.3516x over previous
"""LongcatMoe Trainium2 kernel — expert-parallel sparse MoE across 8 NeuronCores.

Strategy (expert-parallel, per the sharding hint):
  - Host computes the tiny router (fp64 softmax/top-k) and dispatches tokens
    by top-k expert id: core e receives the tokens routed to expert e (padded
    to capacity C=256, capacity factor 1.0; overflow falls back to an exact
    host computation), plus expert e's weights.
  - Each core runs the silu-gated MLP for its expert on its token block.
  - Host combines: out[tok] += gate_weight * y, plus the zero-expert
    (identity) term zero_w[t] * x[t].

v6 kernel: fp8 DoubleRow everywhere, no transposes, one DMA stream.
  Both phases contract over the partition dim with DoubleRow (K=256 per
  step, 2 fp8 rows/cycle):
    Phase 1 (gate/up): stationary w{g,u} blocks [128, 2, 128], moving
      x8 [128, 2, C] -> mid[j*128+p, c] accumulates in PSUM [128, C] —
      mid lands directly in [I, C] layout, no transpose needed.
    Phase 2 (down):   stationary w_down blocks [128, 2, 128], moving
      mid8 [128, 2, C] -> y[k] PSUM [128, C].
  All quantization scales are exact powers of two:
      x *= 2^4, w_gate/w_up *= 2^7          (fp8 e4m3, normal range)
      silu via activation(scale=2^-11) recovers the true gate
      mid8 = (silu * u_psum) * 2^-7          (= mid * 2^4, fp8)
      w_down *= 2^7; y = psum * 2^-11        (copy-with-scale on ACT)
  Rel-err vs reference (fixed seed, host-simulated == HW measured):
  ~1.92e-2, under the 2e-2 gate.

DMA design (v2-v5 trace analysis): DMA transfers serialize in global
semaphore-rotation order, which follows the scheduler's simulated start
times — any late-issued transfer (e.g. a mid transpose) that lands between
weight transfers in that rotation blocks them. v6 therefore has NO
transposes and exactly ONE stream on the SP ring in consumption order:
x8, 8x512KB wgu chunks (j-major), 2x1MB wd chunks, 4x256KB y writes.
The ACT ring carries no DMAs at all.

Host-side layouts (per-partition contiguous for every device DMA):
  x8   [128, KH, 2, C] fp8   x8[p,kh,o,c]      = x[idx[c], kh*256+o*128+p]*2^4
  wgu8 [IO, 128, 2, KH, 2, 128] fp8
                             wgu8[j,p,g,kh,o,m] = w{g}[kh*256+o*128+p,
                                                       j*128+m]*2^7
  wd8  [2, 128, 8, 4, 2, 128] fp8
                             wd8[s,p,k,ip,o,c] = w_down[ip*256+o*128+p,
                                                        (8s+k)*128+c]*2^7
  y    [4, 128, 4, C] bf16 output; host reassembles [H, C]
"""

import os

import numpy as np
import ml_dtypes

T, H, I, E, Z, TOPK = 1024, 2048, 1024, 8, 8, 4
ROUTED_SCALING = 1.0
N_CORES = 8
P = 128
HO = H // P   # 16
IO = I // P   # 8
KH = H // 256  # 8 DoubleRow k-groups (phase 1)
IP = I // 256  # 4 DoubleRow k-groups (phase 2)
C = 256       # per-expert token capacity (capacity factor 1.0; overflow → host)
NWARM = 16    # PE warmup matmuls (bridge HAM until first real matmul)

XS = 2.0 ** 4    # x fp8 pre-scale
WS = 2.0 ** 7    # w_gate/w_up/w_down fp8 pre-scale
SSIL = 2.0 ** -11  # silu input scale (1/(XS*WS))
SMID = 2.0 ** -7   # mid8 = silu*u_psum*SMID = mid*2^4
SY = 2.0 ** -11    # y = psum/(2^4 * 2^7)

_PROGRAM = None
LAST_RESULTS = None  # BassKernelResults of the most recent run (for test harness)


def _build_program():
    import concourse.mybir as mybir
    import concourse.tile as tile
    from concourse import bacc

    f32 = mybir.dt.float32
    bf16 = mybir.dt.bfloat16
    fp8 = mybir.dt.float8e4
    SILU = mybir.ActivationFunctionType.Silu
    COPY = mybir.ActivationFunctionType.Copy
    DR = mybir.MatmulPerfMode.DoubleRow

    nc = bacc.Bacc(
        "TRN2",
        target_bir_lowering=False,
        debug=False,
        enable_asserts=False,
        num_devices=N_CORES,
    )
    x8 = nc.dram_tensor("x8", [P, KH, 2, C], fp8, kind="ExternalInput").ap()
    wgu8 = nc.dram_tensor("wgu8", [IO, P, 2, KH, 2, P], fp8,
                          kind="ExternalInput").ap()
    wd8 = nc.dram_tensor("wd8", [2, P, HO // 2, IP, 2, P], fp8,
                         kind="ExternalInput").ap()
    y = nc.dram_tensor("y", [4, P, 4, C], bf16, kind="ExternalOutput").ap()

    with tile.TileContext(nc) as tc:
        with (
            tc.tile_pool(name="px", bufs=1) as px,
            tc.tile_pool(name="pwgu", bufs=IO) as pwgu,
            tc.tile_pool(name="pwd", bufs=2) as pwd,
            tc.tile_pool(name="pmidf", bufs=2) as pmidf,
            tc.tile_pool(name="pmid8", bufs=1) as pmid8,
            tc.tile_pool(name="psg", bufs=2) as psg,
            tc.tile_pool(name="py", bufs=4) as py,
            tc.tile_pool(name="pwrm", bufs=1) as pwrm,
            tc.tile_pool(name="ppg", bufs=2, space="PSUM") as ppg,
            tc.tile_pool(name="ppu", bufs=2, space="PSUM") as ppu,
            tc.tile_pool(name="ppd", bufs=2, space="PSUM") as ppd,
            tc.tile_pool(name="ppw", bufs=1, space="PSUM") as ppw,
        ):
            # PE warmup: keep the tensor engine busy while the head DMAs land
            # so the HAM clock-gate reaches 2.4 GHz by the first real matmul.
            wtile = pwrm.tile([P, C], bf16)
            nc.vector.memset(wtile[:], 0.0)
            pwm = ppw.tile([P, C], f32)
            for w in range(NWARM):
                nc.tensor.matmul(pwm[:], wtile[:, :P], wtile[:],
                                 start=(w == 0), stop=(w == NWARM - 1))

            xt = px.tile([P, KH, 2, C], fp8)
            wgu_t = [pwgu.tile([P, 2, KH, 2, P], fp8, name=f"wgu{j}",
                               tag="wgu") for j in range(IO)]
            wd_t = [pwd.tile([P, HO // 2, IP, 2, P], fp8, name=f"wd{s}",
                             tag="wd") for s in range(2)]
            mid8 = pmid8.tile([P, IO, C], fp8)

            # ONE DMA stream on the SP ring in exact consumption order.
            with tc.high_priority():
                nc.sync.dma_start(xt[:], x8[:])
                for j in range(IO):
                    nc.sync.dma_start(wgu_t[j][:], wgu8[j])
                for s in range(2):
                    nc.sync.dma_start(wd_t[s][:], wd8[s])

            # Phase 1: mid[j] = silu(x @ Wg_j) * (x @ Wu_j) in [I, C] layout,
            # fp8 DoubleRow with stationary weight blocks.
            for j in range(IO):
                pg = ppg.tile([P, C], f32)
                pu = ppu.tile([P, C], f32)
                for kh in range(KH):
                    nc.tensor.matmul(
                        pg[:], wgu_t[j][:, 0, kh, :, :], xt[:, kh, :, :],
                        start=(kh == 0), stop=(kh == KH - 1),
                        perf_mode=DR,
                    )
                for kh in range(KH):
                    nc.tensor.matmul(
                        pu[:], wgu_t[j][:, 1, kh, :, :], xt[:, kh, :, :],
                        start=(kh == 0), stop=(kh == KH - 1),
                        perf_mode=DR,
                    )
                sg = psg.tile([P, C], f32)
                nc.scalar.activation(sg[:], pg[:], SILU, scale=SSIL)
                midf = pmidf.tile([P, C], f32)
                nc.vector.tensor_mul(out=midf[:], in0=sg[:], in1=pu[:])
                nc.scalar.activation(mid8[:, j, :], midf[:], COPY, scale=SMID)

            # Phase 2: y[k] = mid @ Wd[k], fp8 DoubleRow, [H, C] layout;
            # y tiles written out in groups of 4 on the SP ring.
            for kq in range(4):
                ty = py.tile([P, 4, C], bf16)
                for sub in range(4):
                    k = kq * 4 + sub
                    pd = ppd.tile([P, C], f32)
                    for ip in range(IP):
                        nc.tensor.matmul(
                            pd[:],
                            wd_t[k // (HO // 2)][:, k % (HO // 2), ip, :, :],
                            mid8[:, 2 * ip:2 * ip + 2, :],
                            start=(ip == 0), stop=(ip == IP - 1),
                            perf_mode=DR,
                        )
                    nc.scalar.activation(ty[:, sub, :], pd[:], COPY, scale=SY)
                nc.sync.dma_start(y[kq], ty[:])

    nc.compile()
    return nc


def _route(x, router_w, corr_bias):
    """fp64 router: returns (topk_idx [T,K], topk_w [T,K])."""
    xl = x.astype(np.float64)
    logits = xl @ router_w.astype(np.float64).T
    logits -= logits.max(axis=1, keepdims=True)
    p = np.exp(logits)
    p /= p.sum(axis=1, keepdims=True)
    sel = p + corr_bias.astype(np.float64)
    topk_idx = np.argsort(-sel, axis=1, kind="stable")[:, :TOPK]
    topk_w = np.take_along_axis(p, topk_idx, axis=1) * ROUTED_SCALING
    return topk_idx, topk_w


def kernel(hidden_states, router_w, corr_bias, w_gate, w_up, w_down):
    global _PROGRAM, LAST_RESULTS
    x = np.asarray(hidden_states, dtype=np.float32)
    router_w = np.asarray(router_w, dtype=np.float32)
    corr_bias = np.asarray(corr_bias, dtype=np.float32)
    w_gate = np.asarray(w_gate, dtype=np.float32)
    w_up = np.asarray(w_up, dtype=np.float32)
    w_down = np.asarray(w_down, dtype=np.float32)

    topk_idx, topk_w = _route(x, router_w, corr_bias)
    routed = topk_idx < E
    zero_w = (topk_w * (~routed)).sum(axis=1)  # [T] fp64

    e4 = ml_dtypes.float8_e4m3

    # Dispatch: token list + gate weight per expert; overflow beyond C
    # falls back to an exact host computation.
    idx_list, w_list, overflow = [], [], []
    for e in range(E):
        toks, kpos = np.nonzero(topk_idx == e)
        we = topk_w[toks, kpos]
        if len(toks) > C:
            overflow.append((e, toks[C:], we[C:]))
            toks, we = toks[:C], we[:C]
        idx_list.append(toks)
        w_list.append(we)

    in_maps = []
    for e in range(E):
        toks = idx_list[e]
        n = len(toks)
        xg = np.zeros((C, H), dtype=np.float32)
        xg[:n] = x[toks]
        # x8[p, kh, o, c] = x[c, kh*256+o*128+p] * XS
        x8d = np.ascontiguousarray(
            (xg * XS).astype(e4).reshape(C, KH, 2, P).transpose(3, 1, 2, 0))
        # wgu8[j, p, g, kh, o, m] = w{g}[kh*256+o*128+p, j*128+m] * WS
        wg8 = (w_gate[e] * WS).astype(e4)
        wu8 = (w_up[e] * WS).astype(e4)
        # [g, H, I] -> [g, kh, o, p, j, m] -> [j, p, g, kh, o, m]
        wgu_s = np.stack([wg8, wu8], axis=0).reshape(2, KH, 2, P, IO, P)
        wgud = np.ascontiguousarray(wgu_s.transpose(4, 3, 0, 1, 2, 5))
        # wd8[s, p, k, ip, o, c] = w_down[ip*256+o*128+p, (8s+k)*128+c] * WS
        # [I, H] -> [ip, o, p, s, k, c] -> [s, p, k, ip, o, c]
        wd_s = (w_down[e] * WS).astype(e4).reshape(IP, 2, P, 2, HO // 2, P)
        wdd = np.ascontiguousarray(wd_s.transpose(3, 2, 4, 0, 1, 5))
        in_maps.append({"x8": x8d, "wgu8": wgud, "wd8": wdd})

    if _PROGRAM is None:
        _PROGRAM = _build_program()

    from concourse.bass_utils import run_bass_kernel_spmd

    kw = {}
    if os.environ.get("MOE_KERNEL_TRACE", "") == "1":
        kw = dict(trace=True, trace_cores=list(range(N_CORES)))
    res = run_bass_kernel_spmd(
        _PROGRAM, in_maps, core_ids=list(range(N_CORES)), **kw)
    LAST_RESULTS = res

    out = np.zeros((T, H), dtype=np.float64)
    for e in range(E):
        n = len(idx_list[e])
        if n:
            yr = res.results[e]["y"]  # [4, P, 4, C] bf16
            ye = yr.transpose(0, 2, 1, 3).reshape(H, C)
            out[idx_list[e]] += (w_list[e][:, None]
                                 * ye[:, :n].T.astype(np.float64))
    for e, toks, ws in overflow:
        xt = x[toks].astype(np.float64)
        g = xt @ w_gate[e].astype(np.float64)
        u = xt @ w_up[e].astype(np.float64)
        mid = (g / (1.0 + np.exp(-g))) * u
        out[toks] += ws[:, None] * (mid @ w_down[e].astype(np.float64))
    out += zero_w[:, None] * x.astype(np.float64)
    return out.astype(np.float32)
